# revision 21
# baseline (speedup 1.0000x reference)
"""Trainium2 Bass kernel for nn_DPFlashAttention (B=4, S=2048, E=2048, H=16).

Sharding: 8 cores = 4 batches (data-parallel) x 2 head-groups (tensor-parallel
over heads). Core c handles batch c//2, heads (c%2)*8 .. (c%2)*8+8.

v4: fp8e4m3 DoubleRow matmuls (4 MAC/PE/cycle, 2x the f32r/bf16 rate) for the
q/k/v projections, attention scores, ctx accumulation, and softmax
denominators; bf16 for the output projection (fp8 there would exceed the
2e-2 error budget). Scales: x*32, W*4096, q/k/v re-quantized *16; softmax
computed as exp(score/sqrt(D) - 5) with the offset cancelling in the
normalization (global |score|/sqrt(D) max is 8.92).

Structure: P2 (v projection) runs first while the q/k inputs stream in;
then one loop over heads fuses the per-head q/k projection (P1) with
attention (P3) so the projection's PE work hides under the ACT-bound
softmax exp of the previous head. q/k head tiles go straight to SBUF in
the [64, 2(d-half), S] split layout via partition-shifted DVE converts
(no DRAM scratch roundtrip); the P1 psum tiles share the scores PSUM
pool. P4 (out-projection, bf16) runs last with halved Wo loads
double-buffered across the two DMA queues.

DMA is descriptor-bound on HW: all tensor loads are whole-tensor single
DMAs with 1-4KB contiguous runs per partition line, the output is staged
to [128, S] bf16 tiles (4KB rows), and transfers alternate between the SP
and ACT HWDGE queues.
"""
import math
import sys
from contextlib import ExitStack

sys.path.insert(0, "/opt/trn_rl_repo")

import numpy as np
import ml_dtypes

import concourse.bass as bass
import concourse.mybir as mybir
import concourse.tile as tile
from concourse.vector_clock import ScopedClock


class TileContextFixed(tile.TileContext):
    """This walrus build caps sync waits per instruction; split the closing
    drain's waits across single-wait NoOps (same engine => same semantics)."""

    def _drain_and_barrier(self, tick_clock, wait_clock):
        carrier = self.nc.sync.nop(nofuse=True, hint="drain_waits")
        wait_clock.add_sem_waits(
            carrier.ins, ScopedClock({None: tick_clock.global_clock})
        )
        si = carrier.ins.sync_info
        waits = list(si.on_wait) if si is not None else []
        if si is not None:
            si.on_wait[:] = waits[:1]
        for w in waits[1:]:
            n = self.nc.sync.nop(nofuse=True, hint="drain_waits")
            n.ins.sync_info = mybir.SyncInfo(on_wait=[w], on_update=[])
        self.nc.sync.drain()
        self.nc.all_engine_barrier()
        assert self.sems is not None
        popped = self.nc._tile_sem_poison_stack.pop()
        assert popped is self._sem_poison
        self.nc.clear_and_free_semaphores(list(self.sems.allocated().values()))
        self.nc.all_engine_barrier()


def split_excess_waits(nc, opcodes=None, cap=1):
    """Hoist waits beyond `cap` onto same-engine NoOps placed just before the
    instruction; engine queues execute in order so blocking is preserved."""
    n_split = 0
    for fn in nc.m.functions:
        for blk in fn.blocks:
            new = []
            for inst in blk.instructions:
                si = inst.sync_info
                if (
                    (opcodes is None or inst.opcode in opcodes)
                    and si is not None
                    and len(si.on_wait) > cap
                ):
                    waits = list(si.on_wait)
                    for j, w in enumerate(waits[cap:]):
                        nop = mybir.InstNoOp(
                            name=f"{inst.name}-w{j}", engine=inst.engine
                        )
                        nop.sync_info = mybir.SyncInfo(on_wait=[w], on_update=[])
                        new.append(nop)
                        n_split += 1
                    si.on_wait[:] = waits[:cap]
                new.append(inst)
            blk.instructions[:] = new
    return n_split


def dedupe_ldweights(nc):
    """Convert an InstLdweights into a PE NoOp (keeping its sync_info) when
    the PE weight registers already hold the same stationary tile: same
    AP/offset/memref/perf_mode as the previous load, with only matmults and
    noops on the PE in between. Mirrors walrus's ldw-opt, which is broken in
    this build."""
    n_removed = 0
    for fn in nc.m.functions:
        for blk in fn.blocks:
            last_sig = None
            for i, inst in enumerate(blk.instructions):
                if str(inst.engine) != "EngineType.PE":
                    continue
                t = type(inst).__name__
                if t == "InstLdweights":
                    ap = inst.ins[0]
                    m = getattr(ap, "memref", None)
                    mname = m if isinstance(m, str) else getattr(m, "name", None)
                    sig = (
                        mname,
                        ap.offset,
                        str(ap.ap),
                        str(inst.perf_mode),
                        str(getattr(inst, "is_transpose", None)),
                    )
                    if sig == last_sig:
                        nop = mybir.InstNoOp(
                            name=f"{inst.name}-ldwskip", engine=inst.engine
                        )
                        nop.sync_info = inst.sync_info
                        blk.instructions[i] = nop
                        n_removed += 1
                    else:
                        last_sig = sig
                elif t in ("InstMatmult", "InstNoOp"):
                    continue
                else:
                    last_sig = None
    return n_removed


F32 = mybir.dt.float32
F8 = mybir.dt.float8e4
BF16 = mybir.dt.bfloat16
AF = mybir.ActivationFunctionType
ALU = mybir.AluOpType
DR = mybir.MatmulPerfMode.DoubleRow

S = 2048
E = 2048
EG = 1024          # per-core e_out shard (8 heads x 128)
D = 128
NHEAD = 8          # heads per core
SCALE = 1.0 / math.sqrt(128.0)

NT = 4             # s-chunks of 512
KT = 16            # k-tiles of 128 over E
N512 = 512

# fp8 scaling
SX = 32.0          # input activations
SW = 4096.0        # projection weights
SQ = 16.0          # q/k/v requantization
SXW = SX * SW
QCONV = SQ / SXW   # = 1/8192, psum -> fp8 conversion factor
C_EXP = 5.0        # exp offset; cancels in softmax normalization


def build_kernel_nc(phases=4, split_waits=True, ablate=()):
    ablate = frozenset(ablate)
    nc = bass.Bass()

    xq = nc.dram_tensor("xqT", [E, S], F8, kind="ExternalInput")
    xk = nc.dram_tensor("xkT", [E, S], F8, kind="ExternalInput")
    xv = nc.dram_tensor("xvT", [E, S], F8, kind="ExternalInput")
    wq = nc.dram_tensor("wq", [E, EG], F8, kind="ExternalInput")
    wk = nc.dram_tensor("wk", [E, EG], F8, kind="ExternalInput")
    wv = nc.dram_tensor("wv", [E, EG], F8, kind="ExternalInput")
    wo = nc.dram_tensor("wo", [EG, E], BF16, kind="ExternalInput")
    bq = nc.dram_tensor("bq2", [128, 8], F32, kind="ExternalInput")
    bk = nc.dram_tensor("bk2", [128, 8], F32, kind="ExternalInput")
    bv = nc.dram_tensor("bvb", [128, EG], F32, kind="ExternalInput")
    noi = nc.dram_tensor("noiseT", [EG, S], BF16, kind="ExternalInput")
    out = nc.dram_tensor("outT", [E, S], BF16, kind="ExternalOutput")

    with TileContextFixed(nc) as tc, \
         nc.allow_low_precision(reason="fp8/bf16 matmuls; 2e-2 budget"), \
         ExitStack() as stack:
        cpool = stack.enter_context(tc.tile_pool(name="const", bufs=1))
        bq_sb = cpool.tile([128, 8], F32, tag="bq")
        nc.sync.dma_start(bq_sb[:], bq[:])
        bk_sb = cpool.tile([128, 8], F32, tag="bk")
        nc.sync.dma_start(bk_sb[:], bk[:])
        bv_sb = cpool.tile([128, EG], F32, tag="bv")
        nc.sync.dma_start(bv_sb[:], bv[:])
        # DoubleRow column-sum stationary: value SQ folds the v scale
        # into the denominator so ctx = ps_ctx / ps_den exactly.
        # M=32 because dual-fp8 ldweights rejects column counts < 32;
        # rows 0..31 of the result are identical, row 0 is used.
        ones8 = cpool.tile([128, 2, 32], F8, tag="ones8")
        nc.vector.memset(ones8[:], SQ)
        ones_row = cpool.tile([1, 128], BF16, tag="onesrow")
        nc.vector.memset(ones_row[:], 1.0)
        cexp_sb = cpool.tile([128, 1], F32, tag="cexp")
        nc.vector.memset(cexp_sb[:], -C_EXP)

        vpool = stack.enter_context(tc.tile_pool(name="vres", bufs=1))
        v_sb = vpool.tile([128, KT, EG], F8, tag="vres")

        # ---------------- P2: v projection (natural [s, d]) -------------
        with tc.tile_pool(name="p2w", bufs=1) as wpool, \
             tc.tile_pool(name="p2x", bufs=1) as xpool, \
             tc.tile_pool(name="p2t", bufs=4) as tpool, \
             tc.tile_pool(name="p2ps", bufs=4, space="PSUM") as pspool:
            xv_sb = xpool.tile([128, KT, S], F8, tag="xv")
            nc.sync.dma_start(
                xv_sb[:], xv.rearrange("(kt p) n -> p kt n", p=128)
            )
            wv_sb = wpool.tile([128, KT, EG], F8, tag="wv")
            nc.scalar.dma_start(
                wv_sb[:], wv.rearrange("(kt p) m -> p kt m", p=128)
            )
            for m in range(16):
                pss = [
                    pspool.tile([128, N512], F32, tag=f"psv{nn}",
                                name=f"psv_{m}_{nn}")
                    for nn in range(2)
                ]
                for j in range(8):
                    for nn in range(2):
                        nc.tensor.matmul(
                            pss[nn][:],
                            xv_sb[:, 2 * j:2 * j + 2,
                                  m * 128:(m + 1) * 128],
                            wv_sb[:, 2 * j:2 * j + 2,
                                  nn * N512:(nn + 1) * N512],
                            start=(j == 0),
                            stop=(j == 7),
                            perf_mode=DR,
                        )
                for nn in range(2):
                    tmp = tpool.tile([128, N512], F32, tag="vt")
                    nc.scalar.activation(
                        tmp[:], pss[nn][:], AF.Copy, scale=QCONV
                    )
                    nc.vector.tensor_add(
                        v_sb[:, m, nn * N512:(nn + 1) * N512],
                        tmp[:],
                        bv_sb[:, nn * N512:(nn + 1) * N512],
                    )

        if phases < 2:
            n = split_excess_waits(nc) if split_waits else 0
            return nc, (n, dedupe_ldweights(nc) if split_waits else 0)

        # ------- fused P1+P3: per-head q/k projection + attention -------
        ctxpool = stack.enter_context(tc.tile_pool(name="ctx", bufs=1))
        ctx_sb = ctxpool.tile([128, NHEAD, S], BF16, tag="ctx")
        with tc.tile_pool(name="p1x", bufs=1) as xpool1, \
             tc.tile_pool(name="p1w", bufs=1) as wpool1, \
             tc.tile_pool(name="p3h", bufs=2) as hpool, \
             tc.tile_pool(name="p3p", bufs=2) as ppool, \
             tc.tile_pool(name="p3n", bufs=2) as npool, \
             tc.tile_pool(name="p3s", bufs=1) as spool, \
             tc.tile_pool(name="psS", bufs=2, space="PSUM") as psS, \
             tc.tile_pool(name="psC", bufs=1, space="PSUM") as psC, \
             tc.tile_pool(name="psR", bufs=1, space="PSUM") as psR:
            xq_sb = xpool1.tile([128, KT, S], F8, tag="xq")
            nc.sync.dma_start(
                xq_sb[:], xq.rearrange("(kt p) n -> p kt n", p=128)
            )
            wq_sb = wpool1.tile([128, KT, EG], F8, tag="wq")
            nc.sync.dma_start(
                wq_sb[:], wq.rearrange("(kt p) m -> p kt m", p=128)
            )
            xk_sb = xpool1.tile([128, KT, S], F8, tag="xk")
            nc.scalar.dma_start(
                xk_sb[:], xk.rearrange("(kt p) n -> p kt n", p=128)
            )
            wk_sb = wpool1.tile([128, KT, EG], F8, tag="wk")
            nc.scalar.dma_start(
                wk_sb[:], wk.rearrange("(kt p) m -> p kt m", p=128)
            )

            if "dven" in ablate:
                nc.vector.memset(ctx_sb[:], 0.5)
            psb_shared = None
            if "exp" in ablate:
                psb_shared = ppool.tile([128, 2, 1024], F8, tag="pshare")
                nc.vector.memset(psb_shared[:], 0.25)
            for h in range(NHEAD):
                # ---- P1(h): project q_h, k_h into [64, 2, S] split ----
                qsb = hpool.tile([64, 2, S], F8, tag="qh")
                ksb = hpool.tile([64, 2, S], F8, tag="kh")
                if "p1" in ablate:
                    nc.vector.memset(qsb[:], 0.25)
                    nc.vector.memset(ksb[:], 0.25)
                for (x_sb, w_sb, bsb, dst) in () if "p1" in ablate else (
                    (xq_sb, wq_sb, bq_sb, qsb),
                    (xk_sb, wk_sb, bk_sb, ksb),
                ):
                    pts = [
                        psS.tile([128, 1024], F32, tag="sps",
                                 name=f"p1ps_{h}_{half}")
                        for half in range(2)
                    ]
                    for j in range(8):
                        for half in range(2):
                            for nn in range(2):
                                nc.tensor.matmul(
                                    pts[half][:, nn * N512:(nn + 1) * N512],
                                    w_sb[:, 2 * j:2 * j + 2,
                                         h * 128:(h + 1) * 128],
                                    x_sb[:, 2 * j:2 * j + 2,
                                         half * 1024 + nn * N512:
                                         half * 1024 + (nn + 1) * N512],
                                    start=(j == 0),
                                    stop=(j == 7),
                                    perf_mode=DR,
                                )
                    for half in range(2):
                        # partition-shifted converts into the d-half split
                        nc.vector.tensor_scalar(
                            dst[:, 0, half * 1024:(half + 1) * 1024],
                            pts[half][0:64, :], bsb[0:64, h:h + 1], QCONV,
                            ALU.add, ALU.mult,
                        )
                        nc.vector.tensor_scalar(
                            dst[:, 1, half * 1024:(half + 1) * 1024],
                            pts[half][64:128, :], bsb[64:128, h:h + 1], QCONV,
                            ALU.add, ALU.mult,
                        )

                # ---- P3(h): attention for this head ----
                for qc in range(2):
                    ps_ctx = psC.tile([128, 1024], F32, tag="ctxps")
                    ps_r = psR.tile([128, 1024], F32, tag="sumps")
                    for j in range(8):
                        if psb_shared is not None:
                            psb = psb_shared
                        else:
                            psb = ppool.tile([128, 2, 1024], F8, tag="p")
                        for t in range(2):
                            kt = 2 * j + t
                            ps_s = psS.tile([128, 1024], F32, tag="sps",
                                            name=f"sps_{h}_{qc}_{j}_{t}")
                            for nn in range(2):
                                nc.tensor.matmul(
                                    ps_s[:, nn * N512:(nn + 1) * N512],
                                    ksb[:, :, kt * 128:(kt + 1) * 128],
                                    qsb[:, :,
                                        qc * 1024 + nn * N512:
                                        qc * 1024 + (nn + 1) * N512],
                                    start=True,
                                    stop=True,
                                    perf_mode=DR,
                                )
                            if "exp" not in ablate:
                                nc.scalar.activation(
                                    psb[:, t, :], ps_s[:], AF.Exp,
                                    scale=SCALE / (SQ * SQ),
                                    bias=cexp_sb[:],
                                )
                        for nn in range(2):
                            nc.tensor.matmul(
                                ps_ctx[:, nn * N512:(nn + 1) * N512],
                                v_sb[:, 2 * j:2 * j + 2,
                                     h * 128:(h + 1) * 128],
                                psb[:, :, nn * N512:(nn + 1) * N512],
                                start=(j == 0),
                                stop=(j == 7),
                                perf_mode=DR,
                            )
                        for nn in range(2):
                            nc.tensor.matmul(
                                ps_r[0:32, nn * N512:(nn + 1) * N512],
                                ones8[:],
                                psb[:, :, nn * N512:(nn + 1) * N512],
                                start=(j == 0),
                                stop=(j == 7),
                                perf_mode=DR,
                            )
                    # normalize + noise into resident ctx^T
                    if "dven" in ablate:
                        continue
                    rsb = spool.tile([1, 1024], BF16, tag="r")
                    nc.vector.reciprocal(rsb[:], ps_r[0:1, :])
                    for nn in range(2):
                        nc.tensor.matmul(
                            ps_r[:, nn * N512:(nn + 1) * N512],
                            ones_row[:],
                            rsb[:, nn * N512:(nn + 1) * N512],
                            start=True,
                            stop=True,
                        )
                    nsb = npool.tile([128, 1024], BF16, tag="n")
                    nc.sync.dma_start(
                        nsb[:],
                        noi[h * 128:(h + 1) * 128,
                            qc * 1024:(qc + 1) * 1024],
                    )
                    rb_sb = spool.tile([128, 1024], F32, tag="rb")
                    nc.vector.tensor_copy(rb_sb[:], ps_r[:])
                    tmp = spool.tile([128, 1024], F32, tag="tmp")
                    nc.vector.tensor_mul(tmp[:], ps_ctx[:], rb_sb[:])
                    nc.vector.tensor_add(
                        ctx_sb[:, h, qc * 1024:(qc + 1) * 1024],
                        tmp[:],
                        nsb[:],
                    )

        if phases < 3:
            n = split_excess_waits(nc) if split_waits else 0
            return nc, (n, dedupe_ldweights(nc) if split_waits else 0)

        # ---------------- P4: out projection ------------------------
        with tc.tile_pool(name="p4w", bufs=2) as wpool4, \
             tc.tile_pool(name="p4o", bufs=4) as opool, \
             tc.tile_pool(name="p4ps", bufs=2, space="PSUM") as pspool:
            wo_half = {}
            for g in range(2):
                wh = wpool4.tile([128, NHEAD, 1024], BF16,
                                 tag="woh", name=f"wo_half_{g}")
                eng = nc.sync if g == 0 else nc.scalar
                eng.dma_start(
                    wh[:],
                    wo[:, g * 1024:(g + 1) * 1024]
                    .rearrange("(kt p) n -> p kt n", p=128),
                )
                wo_half[g] = wh
            for m in range(16):
                wh = wo_half[m // 8]
                mm = m % 8
                pss = [
                    pspool.tile([128, N512], F32, tag=f"pso{n}",
                                name=f"pso_{m}_{n}")
                    for n in range(NT)
                ]
                for kt in range(NHEAD):
                    for n in range(NT):
                        nc.tensor.matmul(
                            pss[n][:],
                            wh[:, kt, mm * 128:(mm + 1) * 128],
                            ctx_sb[:, kt, n * N512:(n + 1) * N512],
                            start=(kt == 0),
                            stop=(kt == NHEAD - 1),
                        )
                stage = opool.tile([128, S], BF16, tag="oo")
                for n in range(NT):
                    nc.vector.tensor_copy(
                        stage[:, n * N512:(n + 1) * N512], pss[n][:]
                    )
                eng = nc.sync if m % 2 == 0 else nc.scalar
                eng.dma_start(out[m * 128:(m + 1) * 128, :], stage[:])

    n = split_excess_waits(nc) if split_waits else 0
    nldw = dedupe_ldweights(nc) if split_waits else 0
    return nc, (n, nldw)


B = 4
NOISE_SCALE = 1.0 * math.sqrt(2.0 * math.log(1.25 / 1e-05)) / 1.0


def _fp8(x, scale):
    return np.clip(
        np.asarray(x, np.float32) * scale, -240.0, 240.0
    ).astype(ml_dtypes.float8_e4m3)


def _make_in_maps(query, key_t, value, Wq, bq, Wk, bk, Wv, bv, Wo, bo, noise):
    WqT = np.ascontiguousarray(np.asarray(Wq, np.float32).T)
    WkT = np.ascontiguousarray(np.asarray(Wk, np.float32).T)
    WvT = np.ascontiguousarray(np.asarray(Wv, np.float32).T)
    WoT = np.ascontiguousarray(np.asarray(Wo, np.float32).T)
    bq = np.asarray(bq, np.float32)
    bk = np.asarray(bk, np.float32)
    bv = np.asarray(bv, np.float32)
    xT8 = None
    in_maps = []
    for c in range(8):
        b, g = c // 2, c % 2
        if g == 0:
            xT8 = [
                _fp8(np.asarray(x, np.float32).T, SX)
                for x in (query[b], key_t[b], value[b])
            ]
        cols = slice(g * EG, (g + 1) * EG)
        in_maps.append({
            "xqT": xT8[0],
            "xkT": xT8[1],
            "xvT": xT8[2],
            "wq": _fp8(WqT[:, cols], SW),
            "wk": _fp8(WkT[:, cols], SW),
            "wv": _fp8(WvT[:, cols], SW),
            "wo": np.asarray(WoT[cols, :]).astype(ml_dtypes.bfloat16),
            "bq2": np.ascontiguousarray(bq[cols].reshape(8, 128).T) * SXW,
            "bk2": np.ascontiguousarray(bk[cols].reshape(8, 128).T) * SXW,
            "bvb": np.ascontiguousarray(
                np.broadcast_to(bv[cols][None, :] * SQ, (128, EG))
            ).astype(np.float32),
            "noiseT": (
                np.asarray(noise[b], np.float32)[:, cols].T * NOISE_SCALE
            ).astype(ml_dtypes.bfloat16),
        })
    return in_maps


def kernel(**inputs) -> np.ndarray:
    from concourse.bass_utils import run_bass_kernel_spmd

    nc, _ = build_kernel_nc()
    in_maps = _make_in_maps(**inputs)
    res = run_bass_kernel_spmd(nc, in_maps, core_ids=list(range(8)))
    bo = np.asarray(inputs["bo"], np.float32)
    out = np.empty((B, S, E), np.float32)
    for b in range(B):
        p0 = np.asarray(res.results[2 * b]["outT"], np.float32)
        p1 = np.asarray(res.results[2 * b + 1]["outT"], np.float32)
        out[b] = (p0 + p1).T + bo[None, :]
    return out


# revision 22
# speedup vs baseline: 1.0494x; 1.0494x over previous
"""Trainium2 Bass kernel for nn_DPFlashAttention (B=4, S=2048, E=2048, H=16).

Sharding: 8 cores = 4 batches (data-parallel) x 2 head-groups (tensor-parallel
over heads). Core c handles batch c//2, heads (c%2)*8 .. (c%2)*8+8.

v4: fp8e4m3 DoubleRow matmuls (4 MAC/PE/cycle, 2x the f32r/bf16 rate) for the
q/k/v projections, attention scores, ctx accumulation, and softmax
denominators; bf16 for the output projection (fp8 there would exceed the
2e-2 error budget). Scales: x*32, W*4096, q/k/v re-quantized *16; softmax
computed as exp(score/sqrt(D) - 5) with the offset cancelling in the
normalization (global |score|/sqrt(D) max is 8.92).

Structure: P2 (v projection) runs first while the q/k inputs stream in;
then one loop over heads fuses the per-head q/k projection (P1) with
attention (P3) so the projection's PE work hides under the ACT-bound
softmax exp of the previous head. q/k head tiles go straight to SBUF in
the [64, 2(d-half), S] split layout via partition-shifted DVE converts
(no DRAM scratch roundtrip); the P1 psum tiles share the scores PSUM
pool. P4 (out-projection, bf16) runs last with halved Wo loads
double-buffered across the two DMA queues.

DMA is descriptor-bound on HW: all tensor loads are whole-tensor single
DMAs with 1-4KB contiguous runs per partition line, the output is staged
to [128, S] bf16 tiles (4KB rows), and transfers alternate between the SP
and ACT HWDGE queues.
"""
import math
import sys
from contextlib import ExitStack

sys.path.insert(0, "/opt/trn_rl_repo")

import numpy as np
import ml_dtypes

import concourse.bass as bass
import concourse.mybir as mybir
import concourse.tile as tile
from concourse.vector_clock import ScopedClock


class TileContextFixed(tile.TileContext):
    """This walrus build caps sync waits per instruction; split the closing
    drain's waits across single-wait NoOps (same engine => same semantics)."""

    def _drain_and_barrier(self, tick_clock, wait_clock):
        carrier = self.nc.sync.nop(nofuse=True, hint="drain_waits")
        wait_clock.add_sem_waits(
            carrier.ins, ScopedClock({None: tick_clock.global_clock})
        )
        si = carrier.ins.sync_info
        waits = list(si.on_wait) if si is not None else []
        if si is not None:
            si.on_wait[:] = waits[:1]
        for w in waits[1:]:
            n = self.nc.sync.nop(nofuse=True, hint="drain_waits")
            n.ins.sync_info = mybir.SyncInfo(on_wait=[w], on_update=[])
        self.nc.sync.drain()
        self.nc.all_engine_barrier()
        assert self.sems is not None
        popped = self.nc._tile_sem_poison_stack.pop()
        assert popped is self._sem_poison
        self.nc.clear_and_free_semaphores(list(self.sems.allocated().values()))
        self.nc.all_engine_barrier()


def split_excess_waits(nc, opcodes=None, cap=1):
    """Hoist waits beyond `cap` onto same-engine NoOps placed just before the
    instruction; engine queues execute in order so blocking is preserved."""
    n_split = 0
    for fn in nc.m.functions:
        for blk in fn.blocks:
            new = []
            for inst in blk.instructions:
                si = inst.sync_info
                if (
                    (opcodes is None or inst.opcode in opcodes)
                    and si is not None
                    and len(si.on_wait) > cap
                ):
                    waits = list(si.on_wait)
                    for j, w in enumerate(waits[cap:]):
                        nop = mybir.InstNoOp(
                            name=f"{inst.name}-w{j}", engine=inst.engine
                        )
                        nop.sync_info = mybir.SyncInfo(on_wait=[w], on_update=[])
                        new.append(nop)
                        n_split += 1
                    si.on_wait[:] = waits[:cap]
                new.append(inst)
            blk.instructions[:] = new
    return n_split


def dedupe_ldweights(nc):
    """Convert an InstLdweights into a PE NoOp (keeping its sync_info) when
    the PE weight registers already hold the same stationary tile: same
    AP/offset/memref/perf_mode as the previous load, with only matmults and
    noops on the PE in between. Mirrors walrus's ldw-opt, which is broken in
    this build."""
    n_removed = 0
    for fn in nc.m.functions:
        for blk in fn.blocks:
            last_sig = None
            for i, inst in enumerate(blk.instructions):
                if str(inst.engine) != "EngineType.PE":
                    continue
                t = type(inst).__name__
                if t == "InstLdweights":
                    ap = inst.ins[0]
                    m = getattr(ap, "memref", None)
                    mname = m if isinstance(m, str) else getattr(m, "name", None)
                    sig = (
                        mname,
                        ap.offset,
                        str(ap.ap),
                        str(inst.perf_mode),
                        str(getattr(inst, "is_transpose", None)),
                    )
                    if sig == last_sig:
                        nop = mybir.InstNoOp(
                            name=f"{inst.name}-ldwskip", engine=inst.engine
                        )
                        nop.sync_info = inst.sync_info
                        blk.instructions[i] = nop
                        n_removed += 1
                    else:
                        last_sig = sig
                elif t in ("InstMatmult", "InstNoOp"):
                    continue
                else:
                    last_sig = None
    return n_removed


F32 = mybir.dt.float32
F8 = mybir.dt.float8e4
BF16 = mybir.dt.bfloat16
AF = mybir.ActivationFunctionType
ALU = mybir.AluOpType
DR = mybir.MatmulPerfMode.DoubleRow

S = 2048
E = 2048
EG = 1024          # per-core e_out shard (8 heads x 128)
D = 128
NHEAD = 8          # heads per core
SCALE = 1.0 / math.sqrt(128.0)

NT = 4             # s-chunks of 512
KT = 16            # k-tiles of 128 over E
N512 = 512

# fp8 scaling
SX = 32.0          # input activations
SW = 4096.0        # projection weights
SQ = 16.0          # q/k/v requantization
SXW = SX * SW
QCONV = SQ / SXW   # = 1/8192, psum -> fp8 conversion factor
C_EXP = 5.0        # exp offset; cancels in softmax normalization


def build_kernel_nc(phases=4, split_waits=True, ablate=()):
    ablate = frozenset(ablate)
    nc = bass.Bass()

    xq = nc.dram_tensor("xqT", [E, S], F8, kind="ExternalInput")
    xk = nc.dram_tensor("xkT", [E, S], F8, kind="ExternalInput")
    xv = nc.dram_tensor("xvT", [E, S], F8, kind="ExternalInput")
    wq = nc.dram_tensor("wq", [E, EG], F8, kind="ExternalInput")
    wk = nc.dram_tensor("wk", [E, EG], F8, kind="ExternalInput")
    wv = nc.dram_tensor("wv", [E, EG], F8, kind="ExternalInput")
    wo = nc.dram_tensor("wo", [EG, E], BF16, kind="ExternalInput")
    bq = nc.dram_tensor("bq2", [128, 8], F32, kind="ExternalInput")
    bk = nc.dram_tensor("bk2", [128, 8], F32, kind="ExternalInput")
    bv = nc.dram_tensor("bvb", [128, EG], F32, kind="ExternalInput")
    noi = nc.dram_tensor("noiseT", [EG, S], BF16, kind="ExternalInput")
    out = nc.dram_tensor("outT", [E, S], BF16, kind="ExternalOutput")

    with TileContextFixed(nc) as tc, \
         nc.allow_low_precision(reason="fp8/bf16 matmuls; 2e-2 budget"), \
         ExitStack() as stack:
        cpool = stack.enter_context(tc.tile_pool(name="const", bufs=1))
        bq_sb = cpool.tile([128, 8], F32, tag="bq")
        nc.sync.dma_start(bq_sb[:], bq[:])
        bk_sb = cpool.tile([128, 8], F32, tag="bk")
        nc.sync.dma_start(bk_sb[:], bk[:])
        bv_sb = cpool.tile([128, EG], F32, tag="bv")
        nc.sync.dma_start(bv_sb[:], bv[:])
        # DoubleRow column-sum stationary: value SQ folds the v scale
        # into the denominator so ctx = ps_ctx / ps_den exactly.
        # M=32 because dual-fp8 ldweights rejects column counts < 32;
        # rows 0..31 of the result are identical, row 0 is used.
        ones8 = cpool.tile([128, 2, 32], F8, tag="ones8")
        nc.vector.memset(ones8[:], SQ)
        ones_row = cpool.tile([1, 128], BF16, tag="onesrow")
        nc.vector.memset(ones_row[:], 1.0)
        cexp_sb = cpool.tile([128, 1], F32, tag="cexp")
        nc.vector.memset(cexp_sb[:], -C_EXP)

        vpool = stack.enter_context(tc.tile_pool(name="vres", bufs=1))
        v_sb = vpool.tile([128, KT, EG], F8, tag="vres")

        # ---------------- P2: v projection (natural [s, d]) -------------
        with tc.tile_pool(name="p2w", bufs=1) as wpool, \
             tc.tile_pool(name="p2x", bufs=1) as xpool, \
             tc.tile_pool(name="p2t", bufs=4) as tpool, \
             tc.tile_pool(name="p2ps", bufs=4, space="PSUM") as pspool:
            xv_sb = xpool.tile([128, KT, S], F8, tag="xv")
            nc.sync.dma_start(
                xv_sb[:], xv.rearrange("(kt p) n -> p kt n", p=128)
            )
            wv_sb = wpool.tile([128, KT, EG], F8, tag="wv")
            nc.scalar.dma_start(
                wv_sb[:], wv.rearrange("(kt p) m -> p kt m", p=128)
            )
            for m in range(16):
                pss = [
                    pspool.tile([128, N512], F32, tag=f"psv{nn}",
                                name=f"psv_{m}_{nn}")
                    for nn in range(2)
                ]
                for j in range(8):
                    for nn in range(2):
                        nc.tensor.matmul(
                            pss[nn][:],
                            xv_sb[:, 2 * j:2 * j + 2,
                                  m * 128:(m + 1) * 128],
                            wv_sb[:, 2 * j:2 * j + 2,
                                  nn * N512:(nn + 1) * N512],
                            start=(j == 0),
                            stop=(j == 7),
                            perf_mode=DR,
                        )
                for nn in range(2):
                    tmp = tpool.tile([128, N512], F32, tag="vt")
                    nc.scalar.activation(
                        tmp[:], pss[nn][:], AF.Copy, scale=QCONV
                    )
                    nc.vector.tensor_add(
                        v_sb[:, m, nn * N512:(nn + 1) * N512],
                        tmp[:],
                        bv_sb[:, nn * N512:(nn + 1) * N512],
                    )

        if phases < 2:
            n = split_excess_waits(nc) if split_waits else 0
            return nc, (n, dedupe_ldweights(nc) if split_waits else 0)

        # ------- fused P1+P3: per-head q/k projection + attention -------
        ctxpool = stack.enter_context(tc.tile_pool(name="ctx", bufs=1))
        ctx_sb = ctxpool.tile([128, NHEAD, S], BF16, tag="ctx")
        with tc.tile_pool(name="p1x", bufs=1) as xpool1, \
             tc.tile_pool(name="p1w", bufs=1) as wpool1, \
             tc.tile_pool(name="p3h", bufs=2) as hpool, \
             tc.tile_pool(name="p3p", bufs=2) as ppool, \
             tc.tile_pool(name="p3n", bufs=2) as npool, \
             tc.tile_pool(name="p3s", bufs=1) as spool, \
             tc.tile_pool(name="psS", bufs=2, space="PSUM") as psS, \
             tc.tile_pool(name="psC", bufs=1, space="PSUM") as psC, \
             tc.tile_pool(name="psR", bufs=1, space="PSUM") as psR:
            xq_sb = xpool1.tile([128, KT, S], F8, tag="xq")
            nc.sync.dma_start(
                xq_sb[:], xq.rearrange("(kt p) n -> p kt n", p=128)
            )
            wq_sb = wpool1.tile([128, KT, EG], F8, tag="wq")
            nc.scalar.dma_start(
                wq_sb[:], wq.rearrange("(kt p) m -> p kt m", p=128)
            )
            xk_sb = xpool1.tile([128, KT, S], F8, tag="xk")
            nc.scalar.dma_start(
                xk_sb[:], xk.rearrange("(kt p) n -> p kt n", p=128)
            )
            wk_sb = wpool1.tile([128, KT, EG], F8, tag="wk")
            nc.scalar.dma_start(
                wk_sb[:], wk.rearrange("(kt p) m -> p kt m", p=128)
            )

            if "dven" in ablate:
                nc.vector.memset(ctx_sb[:], 0.5)
            psb_shared = None
            if "exp" in ablate:
                psb_shared = ppool.tile([128, 2, 1024], F8, tag="pshare")
                nc.vector.memset(psb_shared[:], 0.25)
            for h in range(NHEAD):
                # ---- P1(h): project q_h, k_h into [64, 2, S] split ----
                qsb = hpool.tile([64, 2, S], F8, tag="qh")
                ksb = hpool.tile([64, 2, S], F8, tag="kh")
                if "p1" in ablate:
                    nc.vector.memset(qsb[:], 0.25)
                    nc.vector.memset(ksb[:], 0.25)
                for (x_sb, w_sb, bsb, dst) in () if "p1" in ablate else (
                    (xq_sb, wq_sb, bq_sb, qsb),
                    (xk_sb, wk_sb, bk_sb, ksb),
                ):
                    pts = [
                        psS.tile([128, 1024], F32, tag="sps",
                                 name=f"p1ps_{h}_{half}")
                        for half in range(2)
                    ]
                    for j in range(8):
                        for half in range(2):
                            for nn in range(2):
                                nc.tensor.matmul(
                                    pts[half][:, nn * N512:(nn + 1) * N512],
                                    w_sb[:, 2 * j:2 * j + 2,
                                         h * 128:(h + 1) * 128],
                                    x_sb[:, 2 * j:2 * j + 2,
                                         half * 1024 + nn * N512:
                                         half * 1024 + (nn + 1) * N512],
                                    start=(j == 0),
                                    stop=(j == 7),
                                    perf_mode=DR,
                                )
                    for half in range(2):
                        # partition-shifted converts into the d-half split
                        nc.vector.tensor_scalar(
                            dst[:, 0, half * 1024:(half + 1) * 1024],
                            pts[half][0:64, :], bsb[0:64, h:h + 1], QCONV,
                            ALU.add, ALU.mult,
                        )
                        nc.vector.tensor_scalar(
                            dst[:, 1, half * 1024:(half + 1) * 1024],
                            pts[half][64:128, :], bsb[64:128, h:h + 1], QCONV,
                            ALU.add, ALU.mult,
                        )

                # ---- P3(h): attention for this head ----
                for qc in range(2):
                    ps_ctx = psC.tile([128, 1024], F32, tag="ctxps")
                    ps_r = psR.tile([128, 1024], F32, tag="sumps")
                    for j in range(8):
                        if psb_shared is not None:
                            psb = psb_shared
                        else:
                            psb = ppool.tile([128, 2, 1024], F8, tag="p")
                        for t in range(2):
                            kt = 2 * j + t
                            ps_s = psS.tile([128, 1024], F32, tag="sps",
                                            name=f"sps_{h}_{qc}_{j}_{t}")
                            for nn in range(2):
                                nc.tensor.matmul(
                                    ps_s[:, nn * N512:(nn + 1) * N512],
                                    ksb[:, :, kt * 128:(kt + 1) * 128],
                                    qsb[:, :,
                                        qc * 1024 + nn * N512:
                                        qc * 1024 + (nn + 1) * N512],
                                    start=True,
                                    stop=True,
                                    perf_mode=DR,
                                )
                            if "exp" not in ablate:
                                nc.scalar.activation(
                                    psb[:, t, :], ps_s[:], AF.Exp,
                                    scale=SCALE / (SQ * SQ),
                                    bias=cexp_sb[:],
                                )
                        for nn in range(2):
                            nc.tensor.matmul(
                                ps_ctx[:, nn * N512:(nn + 1) * N512],
                                v_sb[:, 2 * j:2 * j + 2,
                                     h * 128:(h + 1) * 128],
                                psb[:, :, nn * N512:(nn + 1) * N512],
                                start=(j == 0),
                                stop=(j == 7),
                                perf_mode=DR,
                            )
                        for nn in range(2):
                            nc.tensor.matmul(
                                ps_r[0:32, nn * N512:(nn + 1) * N512],
                                ones8[:],
                                psb[:, :, nn * N512:(nn + 1) * N512],
                                start=(j == 0),
                                stop=(j == 7),
                                perf_mode=DR,
                            )
                    # normalize + noise into resident ctx^T
                    if "dven" in ablate:
                        continue
                    rsb = spool.tile([1, 1024], BF16, tag="r")
                    nc.vector.reciprocal(rsb[:], ps_r[0:1, :])
                    for nn in range(2):
                        nc.tensor.matmul(
                            ps_r[:, nn * N512:(nn + 1) * N512],
                            ones_row[:],
                            rsb[:, nn * N512:(nn + 1) * N512],
                            start=True,
                            stop=True,
                        )
                    nsb = npool.tile([128, 1024], BF16, tag="n")
                    nc.sync.dma_start(
                        nsb[:],
                        noi[h * 128:(h + 1) * 128,
                            qc * 1024:(qc + 1) * 1024],
                    )
                    rb_sb = spool.tile([128, 1024], F32, tag="rb")
                    nc.vector.tensor_copy(rb_sb[:], ps_r[:])
                    tmp = spool.tile([128, 1024], F32, tag="tmp")
                    nc.vector.tensor_mul(tmp[:], ps_ctx[:], rb_sb[:])
                    nc.vector.tensor_add(
                        ctx_sb[:, h, qc * 1024:(qc + 1) * 1024],
                        tmp[:],
                        nsb[:],
                    )

        if phases < 3:
            n = split_excess_waits(nc) if split_waits else 0
            return nc, (n, dedupe_ldweights(nc) if split_waits else 0)

        # ---------------- P4: out projection ------------------------
        with tc.tile_pool(name="p4w", bufs=2) as wpool4, \
             tc.tile_pool(name="p4o", bufs=4) as opool, \
             tc.tile_pool(name="p4ps", bufs=2, space="PSUM") as pspool:
            wo_half = {}
            for g in range(2):
                wh = wpool4.tile([128, NHEAD, 1024], BF16,
                                 tag="woh", name=f"wo_half_{g}")
                eng = nc.sync if g == 0 else nc.scalar
                eng.dma_start(
                    wh[:],
                    wo[:, g * 1024:(g + 1) * 1024]
                    .rearrange("(kt p) n -> p kt n", p=128),
                )
                wo_half[g] = wh
            for m in range(16):
                wh = wo_half[m // 8]
                mm = m % 8
                pss = [
                    pspool.tile([128, N512], F32, tag=f"pso{n}",
                                name=f"pso_{m}_{n}")
                    for n in range(NT)
                ]
                for kt in range(NHEAD):
                    for n in range(NT):
                        nc.tensor.matmul(
                            pss[n][:],
                            wh[:, kt, mm * 128:(mm + 1) * 128],
                            ctx_sb[:, kt, n * N512:(n + 1) * N512],
                            start=(kt == 0),
                            stop=(kt == NHEAD - 1),
                        )
                stage = opool.tile([128, S], BF16, tag="oo")
                for n in range(NT):
                    nc.vector.tensor_copy(
                        stage[:, n * N512:(n + 1) * N512], pss[n][:]
                    )
                eng = nc.sync if m % 2 == 0 else nc.scalar
                eng.dma_start(out[m * 128:(m + 1) * 128, :], stage[:])

    n = split_excess_waits(nc) if split_waits else 0
    nldw = dedupe_ldweights(nc) if split_waits else 0
    return nc, (n, nldw)


B = 4
NOISE_SCALE = 1.0 * math.sqrt(2.0 * math.log(1.25 / 1e-05)) / 1.0


def _fp8(x, scale):
    return np.clip(
        np.asarray(x, np.float32) * scale, -240.0, 240.0
    ).astype(ml_dtypes.float8_e4m3)


def _make_in_maps(query, key_t, value, Wq, bq, Wk, bk, Wv, bv, Wo, bo, noise):
    WqT = np.ascontiguousarray(np.asarray(Wq, np.float32).T)
    WkT = np.ascontiguousarray(np.asarray(Wk, np.float32).T)
    WvT = np.ascontiguousarray(np.asarray(Wv, np.float32).T)
    WoT = np.ascontiguousarray(np.asarray(Wo, np.float32).T)
    bq = np.asarray(bq, np.float32)
    bk = np.asarray(bk, np.float32)
    bv = np.asarray(bv, np.float32)
    xT8 = None
    in_maps = []
    for c in range(8):
        b, g = c // 2, c % 2
        if g == 0:
            xT8 = [
                _fp8(np.asarray(x, np.float32).T, SX)
                for x in (query[b], key_t[b], value[b])
            ]
        cols = slice(g * EG, (g + 1) * EG)
        in_maps.append({
            "xqT": xT8[0],
            "xkT": xT8[1],
            "xvT": xT8[2],
            "wq": _fp8(WqT[:, cols], SW),
            "wk": _fp8(WkT[:, cols], SW),
            "wv": _fp8(WvT[:, cols], SW),
            "wo": np.asarray(WoT[cols, :]).astype(ml_dtypes.bfloat16),
            "bq2": np.ascontiguousarray(bq[cols].reshape(8, 128).T) * SXW,
            "bk2": np.ascontiguousarray(bk[cols].reshape(8, 128).T) * SXW,
            "bvb": np.ascontiguousarray(
                np.broadcast_to(bv[cols][None, :] * SQ, (128, EG))
            ).astype(np.float32),
            "noiseT": (
                np.asarray(noise[b], np.float32)[:, cols].T * NOISE_SCALE
            ).astype(ml_dtypes.bfloat16),
        })
    return in_maps


def kernel(**inputs) -> np.ndarray:
    from concourse.bass_utils import run_bass_kernel_spmd

    nc, _ = build_kernel_nc()
    in_maps = _make_in_maps(**inputs)
    res = run_bass_kernel_spmd(nc, in_maps, core_ids=list(range(8)))
    bo = np.asarray(inputs["bo"], np.float32)
    out = np.empty((B, S, E), np.float32)
    for b in range(B):
        p0 = np.asarray(res.results[2 * b]["outT"], np.float32)
        p1 = np.asarray(res.results[2 * b + 1]["outT"], np.float32)
        out[b] = (p0 + p1).T + bo[None, :]
    return out


# revision 23
# speedup vs baseline: 1.0627x; 1.0127x over previous
"""Trainium2 Bass kernel for nn_DPFlashAttention (B=4, S=2048, E=2048, H=16).

Sharding: 8 cores = 4 batches (data-parallel) x 2 head-groups (tensor-parallel
over heads). Core c handles batch c//2, heads (c%2)*8 .. (c%2)*8+8.

v3: fp8e4m3 DoubleRow matmuls (4 MAC/PE/cycle, 2x the f32r/bf16 rate) for the
q/k/v projections, attention scores, ctx accumulation, and softmax
denominators; bf16 for the output projection (fp8 there would exceed the
2e-2 error budget). Scales: x*32, W*4096, q/k/v re-quantized *16; softmax
computed as exp(score/sqrt(D) - 5) with the offset cancelling in the
normalization (global |score|/sqrt(D) max is 8.92).

DMA is descriptor-bound on HW: all tensor loads are whole-tensor single
DMAs with 1-4KB contiguous runs per partition line, scratch/output writes
are staged to [128, S]-wide tiles (2-4KB rows), and transfers alternate
between the SP and ACT HWDGE queues.

Per-core pipeline:
  P1  q,k projections -> fp8 DRAM scratch [EG, S]; read back per head as
      [64, 2, S] (d split into halves) so scores run DoubleRow with K=64.
  P2  v projection -> resident SBUF [128, 16kt, EG] fp8.
  P3  per head/query-chunk: scores DoubleRow -> exp on ACT -> fp8 P tiles;
      ctx and denominator DoubleRow accumulation over s_k-tile pairs;
      reciprocal + ones-row broadcast matmul; normalize, add DP noise,
      store ctx^T resident bf16.
  P4  out^T partial = Wo_shard @ ctx^T in bf16.
Host: pre-transposes/quantizes inputs, pre-scales noise by the DP sigma,
sums head-group partials, transposes back, adds bo.
"""
import math
import sys
from contextlib import ExitStack

sys.path.insert(0, "/opt/trn_rl_repo")

import numpy as np
import ml_dtypes

import concourse.bass as bass
import concourse.mybir as mybir
import concourse.tile as tile
from concourse.vector_clock import ScopedClock


class TileContextFixed(tile.TileContext):
    """This walrus build caps sync waits per instruction; split the closing
    drain's waits across single-wait NoOps (same engine => same semantics)."""

    def _drain_and_barrier(self, tick_clock, wait_clock):
        carrier = self.nc.sync.nop(nofuse=True, hint="drain_waits")
        wait_clock.add_sem_waits(
            carrier.ins, ScopedClock({None: tick_clock.global_clock})
        )
        si = carrier.ins.sync_info
        waits = list(si.on_wait) if si is not None else []
        if si is not None:
            si.on_wait[:] = waits[:1]
        for w in waits[1:]:
            n = self.nc.sync.nop(nofuse=True, hint="drain_waits")
            n.ins.sync_info = mybir.SyncInfo(on_wait=[w], on_update=[])
        self.nc.sync.drain()
        self.nc.all_engine_barrier()
        assert self.sems is not None
        popped = self.nc._tile_sem_poison_stack.pop()
        assert popped is self._sem_poison
        self.nc.clear_and_free_semaphores(list(self.sems.allocated().values()))
        self.nc.all_engine_barrier()


def split_excess_waits(nc, opcodes=None, cap=1):
    """Hoist waits beyond `cap` onto same-engine NoOps placed just before the
    instruction; engine queues execute in order so blocking is preserved."""
    n_split = 0
    for fn in nc.m.functions:
        for blk in fn.blocks:
            new = []
            for inst in blk.instructions:
                si = inst.sync_info
                if (
                    (opcodes is None or inst.opcode in opcodes)
                    and si is not None
                    and len(si.on_wait) > cap
                ):
                    waits = list(si.on_wait)
                    for j, w in enumerate(waits[cap:]):
                        nop = mybir.InstNoOp(
                            name=f"{inst.name}-w{j}", engine=inst.engine
                        )
                        nop.sync_info = mybir.SyncInfo(on_wait=[w], on_update=[])
                        new.append(nop)
                        n_split += 1
                    si.on_wait[:] = waits[:cap]
                new.append(inst)
            blk.instructions[:] = new
    return n_split


F32 = mybir.dt.float32
F8 = mybir.dt.float8e4
BF16 = mybir.dt.bfloat16
AF = mybir.ActivationFunctionType
ALU = mybir.AluOpType
DR = mybir.MatmulPerfMode.DoubleRow

S = 2048
E = 2048
EG = 1024          # per-core e_out shard (8 heads x 128)
D = 128
NHEAD = 8          # heads per core
SCALE = 1.0 / math.sqrt(128.0)

NT = 4             # s-chunks of 512 in projections / out-proj
KT = 16            # k-tiles of 128 over E
N512 = 512

# fp8 scaling
SX = 32.0          # input activations
SW = 4096.0        # projection weights
SQ = 16.0          # q/k/v requantization
SXW = SX * SW
QCONV = SQ / SXW   # = 1/8192, psum -> fp8 conversion factor
C_EXP = 5.0        # exp offset; cancels in softmax normalization


def build_kernel_nc(phases=4, split_waits=True):
    nc = bass.Bass()

    xq = nc.dram_tensor("xqT", [E, S], F8, kind="ExternalInput")
    xk = nc.dram_tensor("xkT", [E, S], F8, kind="ExternalInput")
    xv = nc.dram_tensor("xvT", [E, S], F8, kind="ExternalInput")
    wq = nc.dram_tensor("wq", [E, EG], F8, kind="ExternalInput")
    wk = nc.dram_tensor("wk", [E, EG], F8, kind="ExternalInput")
    wv = nc.dram_tensor("wv", [E, EG], F8, kind="ExternalInput")
    wo = nc.dram_tensor("wo", [EG, E], BF16, kind="ExternalInput")
    bq = nc.dram_tensor("bq2", [128, 8], F32, kind="ExternalInput")
    bk = nc.dram_tensor("bk2", [128, 8], F32, kind="ExternalInput")
    bv = nc.dram_tensor("bvb", [128, EG], F32, kind="ExternalInput")
    noi = nc.dram_tensor("noiseT", [EG, S], BF16, kind="ExternalInput")
    out = nc.dram_tensor("outT", [E, S], BF16, kind="ExternalOutput")

    qT = nc.dram_tensor("qT_scr", [EG, S], F8, kind="Internal")
    kTd = nc.dram_tensor("kT_scr", [EG, S], F8, kind="Internal")

    with TileContextFixed(nc) as tc, \
         nc.allow_low_precision(reason="fp8/bf16 matmuls; 2e-2 budget"), \
         ExitStack() as stack:
        cpool = stack.enter_context(tc.tile_pool(name="const", bufs=1))
        bq_sb = cpool.tile([128, 8], F32, tag="bq")
        nc.sync.dma_start(bq_sb[:], bq[:])
        bk_sb = cpool.tile([128, 8], F32, tag="bk")
        nc.sync.dma_start(bk_sb[:], bk[:])
        bv_sb = cpool.tile([128, EG], F32, tag="bv")
        nc.sync.dma_start(bv_sb[:], bv[:])
        # DoubleRow column-sum stationary: value SQ folds the v scale
        # into the denominator so ctx = ps_ctx / ps_den exactly.
        # M=32 because dual-fp8 ldweights rejects column counts < 32;
        # rows 0..31 of the result are identical, row 0 is used.
        ones8 = cpool.tile([128, 2, 32], F8, tag="ones8")
        nc.vector.memset(ones8[:], SQ)
        ones_row = cpool.tile([1, 128], BF16, tag="onesrow")
        nc.vector.memset(ones_row[:], 1.0)
        cexp_sb = cpool.tile([128, 1], F32, tag="cexp")
        nc.vector.memset(cexp_sb[:], -C_EXP)

        # ---------------- P1: q/k projections (feature-major out) -------
        # Whole-tensor loads; stationary w (m,j)-slice streams all 4
        # n-chunks (4 PSUM banks) before switching.
        with tc.tile_pool(name="p1w", bufs=1) as wpool, \
             tc.tile_pool(name="p1x", bufs=1) as xpool, \
             tc.tile_pool(name="p1o", bufs=4) as opool, \
             tc.tile_pool(name="p1ps", bufs=2, space="PSUM") as pspool:
            xq_sb = xpool.tile([128, KT, S], F8, tag="xq")
            nc.sync.dma_start(
                xq_sb[:], xq.rearrange("(kt p) n -> p kt n", p=128)
            )
            wq_sb = wpool.tile([128, KT, EG], F8, tag="wq")
            nc.sync.dma_start(
                wq_sb[:], wq.rearrange("(kt p) m -> p kt m", p=128)
            )
            xk_sb = xpool.tile([128, KT, S], F8, tag="xk")
            nc.scalar.dma_start(
                xk_sb[:], xk.rearrange("(kt p) n -> p kt n", p=128)
            )
            wk_sb = wpool.tile([128, KT, EG], F8, tag="wk")
            nc.scalar.dma_start(
                wk_sb[:], wk.rearrange("(kt p) m -> p kt m", p=128)
            )
            for (x_sb, w_sb, bsb, tdst, dmaeng) in (
                (xq_sb, wq_sb, bq_sb, qT, nc.sync),
                (xk_sb, wk_sb, bk_sb, kTd, nc.scalar),
            ):
                for m in range(8):
                    pss = [
                        pspool.tile([128, N512], F32, tag=f"ps{n}",
                                    name=f"ps_{m}_{n}")
                        for n in range(NT)
                    ]
                    for j in range(8):
                        for n in range(NT):
                            nc.tensor.matmul(
                                pss[n][:],
                                w_sb[:, 2 * j:2 * j + 2,
                                     m * 128:(m + 1) * 128],
                                x_sb[:, 2 * j:2 * j + 2,
                                     n * N512:(n + 1) * N512],
                                start=(j == 0),
                                stop=(j == 7),
                                perf_mode=DR,
                            )
                    stage = opool.tile([128, S], F8, tag="o")
                    for n in range(NT):
                        nc.vector.tensor_scalar(
                            stage[:, n * N512:(n + 1) * N512], pss[n][:],
                            bsb[:, m:m + 1], QCONV, ALU.add, ALU.mult,
                        )
                    dmaeng.dma_start(
                        tdst[m * 128:(m + 1) * 128, :], stage[:]
                    )

        # ---------------- P2: v projection (natural [s, d]) -------------
        if phases >= 2:
            hpool = stack.enter_context(tc.tile_pool(name="p3h", bufs=2))
            vpool = stack.enter_context(tc.tile_pool(name="p3v", bufs=1))
            v_sb = vpool.tile([128, KT, EG], F8, tag="vres")
            with tc.tile_pool(name="p2w", bufs=1) as wpool, \
                 tc.tile_pool(name="p2x", bufs=1) as xpool, \
                 tc.tile_pool(name="p2t", bufs=4) as tpool, \
                 tc.tile_pool(name="p2ps", bufs=4, space="PSUM") as pspool:
                xv_sb = xpool.tile([128, KT, S], F8, tag="xv")
                nc.sync.dma_start(
                    xv_sb[:], xv.rearrange("(kt p) n -> p kt n", p=128)
                )
                wv_sb = wpool.tile([128, KT, EG], F8, tag="wv")
                nc.scalar.dma_start(
                    wv_sb[:], wv.rearrange("(kt p) m -> p kt m", p=128)
                )
                for m in range(16):
                    pss = [
                        pspool.tile([128, N512], F32, tag=f"psv{nn}",
                                    name=f"psv_{m}_{nn}")
                        for nn in range(2)
                    ]
                    for j in range(8):
                        for nn in range(2):
                            nc.tensor.matmul(
                                pss[nn][:],
                                xv_sb[:, 2 * j:2 * j + 2,
                                      m * 128:(m + 1) * 128],
                                wv_sb[:, 2 * j:2 * j + 2,
                                      nn * N512:(nn + 1) * N512],
                                start=(j == 0),
                                stop=(j == 7),
                                perf_mode=DR,
                            )
                    for nn in range(2):
                        tmp = tpool.tile([128, N512], F32, tag="vt")
                        nc.scalar.activation(
                            tmp[:], pss[nn][:], AF.Copy, scale=QCONV
                        )
                        nc.vector.tensor_add(
                            v_sb[:, m, nn * N512:(nn + 1) * N512],
                            tmp[:],
                            bv_sb[:, nn * N512:(nn + 1) * N512],
                        )

        # ---------------- P3: attention, resident ctx^T -----------------
        if phases >= 3:
            ctxpool = stack.enter_context(tc.tile_pool(name="ctx", bufs=1))
            ctx_sb = ctxpool.tile([128, NHEAD, S], BF16, tag="ctx")
            wpool4 = stack.enter_context(tc.tile_pool(name="p4w", bufs=1))
            wo_sb = wpool4.tile([128, NHEAD, E], BF16, tag="wo")
            nc.scalar.dma_start(
                wo_sb[:], wo.rearrange("(kt p) n -> p kt n", p=128)
            )
            with tc.tile_pool(name="p3p", bufs=2) as ppool, \
                 tc.tile_pool(name="p3n", bufs=2) as npool, \
                 tc.tile_pool(name="p3s", bufs=1) as spool, \
                 tc.tile_pool(name="psS", bufs=2, space="PSUM") as psS, \
                 tc.tile_pool(name="psC", bufs=1, space="PSUM") as psC, \
                 tc.tile_pool(name="psR", bufs=1, space="PSUM") as psR:
                for h in range(NHEAD):
                    qsb = hpool.tile([64, 2, S], F8, tag="qh")
                    nc.sync.dma_start(
                        qsb[:],
                        qT[h * 128:(h + 1) * 128, :]
                        .rearrange("(two p) s -> p two s", p=64),
                    )
                    ksb = hpool.tile([64, 2, S], F8, tag="kh")
                    nc.sync.dma_start(
                        ksb[:],
                        kTd[h * 128:(h + 1) * 128, :]
                        .rearrange("(two p) s -> p two s", p=64),
                    )
                    for qc in range(2):
                        ps_ctx = psC.tile([128, 1024], F32, tag="ctxps")
                        ps_r = psR.tile([128, 1024], F32, tag="sumps")
                        for j in range(8):
                            psb = ppool.tile([128, 2, 1024], F8, tag="p")
                            for t in range(2):
                                kt = 2 * j + t
                                ps_s = psS.tile([128, 1024], F32, tag="sps")
                                for nn in range(2):
                                    nc.tensor.matmul(
                                        ps_s[:, nn * N512:(nn + 1) * N512],
                                        ksb[:, :, kt * 128:(kt + 1) * 128],
                                        qsb[:, :,
                                            qc * 1024 + nn * N512:
                                            qc * 1024 + (nn + 1) * N512],
                                        start=True,
                                        stop=True,
                                        perf_mode=DR,
                                    )
                                nc.scalar.activation(
                                    psb[:, t, :], ps_s[:], AF.Exp,
                                    scale=SCALE / (SQ * SQ),
                                    bias=cexp_sb[:],
                                )
                            for nn in range(2):
                                nc.tensor.matmul(
                                    ps_ctx[:, nn * N512:(nn + 1) * N512],
                                    v_sb[:, 2 * j:2 * j + 2,
                                         h * 128:(h + 1) * 128],
                                    psb[:, :, nn * N512:(nn + 1) * N512],
                                    start=(j == 0),
                                    stop=(j == 7),
                                    perf_mode=DR,
                                )
                                nc.tensor.matmul(
                                    ps_r[0:32, nn * N512:(nn + 1) * N512],
                                    ones8[:],
                                    psb[:, :, nn * N512:(nn + 1) * N512],
                                    start=(j == 0),
                                    stop=(j == 7),
                                    perf_mode=DR,
                                )
                        # normalize + noise into resident ctx^T
                        rsb = spool.tile([1, 1024], BF16, tag="r")
                        nc.vector.reciprocal(rsb[:], ps_r[0:1, :])
                        for nn in range(2):
                            nc.tensor.matmul(
                                ps_r[:, nn * N512:(nn + 1) * N512],
                                ones_row[:],
                                rsb[:, nn * N512:(nn + 1) * N512],
                                start=True,
                                stop=True,
                            )
                        nsb = npool.tile([128, 1024], BF16, tag="n")
                        nc.sync.dma_start(
                            nsb[:],
                            noi[h * 128:(h + 1) * 128,
                                qc * 1024:(qc + 1) * 1024],
                        )
                        rb_sb = spool.tile([128, 1024], F32, tag="rb")
                        nc.vector.tensor_copy(rb_sb[:], ps_r[:])
                        tmp = spool.tile([128, 1024], F32, tag="tmp")
                        nc.vector.tensor_mul(tmp[:], ps_ctx[:], rb_sb[:])
                        nc.vector.tensor_add(
                            ctx_sb[:, h, qc * 1024:(qc + 1) * 1024],
                            tmp[:],
                            nsb[:],
                        )

        # ---------------- P4: out projection ------------------------
        if phases >= 4:
            with tc.tile_pool(name="p4o", bufs=4) as opool, \
                 tc.tile_pool(name="p4ps", bufs=2, space="PSUM") as pspool:
                for m in range(16):
                    pss = [
                        pspool.tile([128, N512], F32, tag=f"pso{n}",
                                    name=f"pso_{m}_{n}")
                        for n in range(NT)
                    ]
                    for kt in range(NHEAD):
                        for n in range(NT):
                            nc.tensor.matmul(
                                pss[n][:],
                                wo_sb[:, kt, m * 128:(m + 1) * 128],
                                ctx_sb[:, kt, n * N512:(n + 1) * N512],
                                start=(kt == 0),
                                stop=(kt == NHEAD - 1),
                            )
                    stage = opool.tile([128, S], BF16, tag="oo")
                    for n in range(NT):
                        nc.vector.tensor_copy(
                            stage[:, n * N512:(n + 1) * N512], pss[n][:]
                        )
                    eng = nc.sync if m % 2 == 0 else nc.scalar
                    eng.dma_start(out[m * 128:(m + 1) * 128, :], stage[:])

    n = split_excess_waits(nc) if split_waits else 0
    return nc, n


B = 4
NOISE_SCALE = 1.0 * math.sqrt(2.0 * math.log(1.25 / 1e-05)) / 1.0


def _fp8(x, scale):
    return np.clip(
        np.asarray(x, np.float32) * scale, -240.0, 240.0
    ).astype(ml_dtypes.float8_e4m3)


def _make_in_maps(query, key_t, value, Wq, bq, Wk, bk, Wv, bv, Wo, bo, noise):
    WqT = np.ascontiguousarray(np.asarray(Wq, np.float32).T)
    WkT = np.ascontiguousarray(np.asarray(Wk, np.float32).T)
    WvT = np.ascontiguousarray(np.asarray(Wv, np.float32).T)
    WoT = np.ascontiguousarray(np.asarray(Wo, np.float32).T)
    bq = np.asarray(bq, np.float32)
    bk = np.asarray(bk, np.float32)
    bv = np.asarray(bv, np.float32)
    xT8 = None
    in_maps = []
    for c in range(8):
        b, g = c // 2, c % 2
        if g == 0:
            xT8 = [
                _fp8(np.asarray(x, np.float32).T, SX)
                for x in (query[b], key_t[b], value[b])
            ]
        cols = slice(g * EG, (g + 1) * EG)
        in_maps.append({
            "xqT": xT8[0],
            "xkT": xT8[1],
            "xvT": xT8[2],
            "wq": _fp8(WqT[:, cols], SW),
            "wk": _fp8(WkT[:, cols], SW),
            "wv": _fp8(WvT[:, cols], SW),
            "wo": np.asarray(WoT[cols, :]).astype(ml_dtypes.bfloat16),
            "bq2": np.ascontiguousarray(bq[cols].reshape(8, 128).T) * SXW,
            "bk2": np.ascontiguousarray(bk[cols].reshape(8, 128).T) * SXW,
            "bvb": np.ascontiguousarray(
                np.broadcast_to(bv[cols][None, :] * SQ, (128, EG))
            ).astype(np.float32),
            "noiseT": (
                np.asarray(noise[b], np.float32)[:, cols].T * NOISE_SCALE
            ).astype(ml_dtypes.bfloat16),
        })
    return in_maps


def kernel(**inputs) -> np.ndarray:
    from concourse.bass_utils import run_bass_kernel_spmd

    nc, _ = build_kernel_nc()
    in_maps = _make_in_maps(**inputs)
    res = run_bass_kernel_spmd(nc, in_maps, core_ids=list(range(8)))
    bo = np.asarray(inputs["bo"], np.float32)
    out = np.empty((B, S, E), np.float32)
    for b in range(B):
        p0 = np.asarray(res.results[2 * b]["outT"], np.float32)
        p1 = np.asarray(res.results[2 * b + 1]["outT"], np.float32)
        out[b] = (p0 + p1).T + bo[None, :]
    return out
